# revision 21
# baseline (speedup 1.0000x reference)
"""Trainium2 Bass kernel for: freq-domain Butterworth mask -> 3x3 conv ->
BatchNorm(train) -> SiLU, concat(act, freq).

Sharding: data-parallel over batch (2 images per core, 8 cores). BN statistics
are all-reduced across cores with an in-kernel AllReduce collective.

Layout: x arrives f16 (host-cast). Both local images live on SBUF partitions
0-63 with an image slot in the free dim: x2[0:64, i] = img i channels
(unshifted); one SBUF->SBUF DMA fills x2[64:128, i] = img i shifted left one
column. One K=128 matmul then computes TWO conv taps (dx=0 lower rows + dx=1
upper rows), and the two images run as concurrent column-tile matmuls
(tile_position (0,0)/(0,64)) into separate PSUM partition halves:
9 taps/round -> 6 matmul slots/round. Conv runs in f16 (full PE rate, fp32
PSUM accumulate). Conv output y stays resident in SBUF as f16 between the
stats pass and the normalization pass; BN stats use bn_stats/bn_aggr and one
tiny AllReduce. freq = x*dm is written out through an f16->f32 casting SWDGE
DMA (~7e-4 rel, well under the tolerance).
"""

import numpy as np

B, C, H, W = 16, 64, 256, 256
N_CORES = 8
B_LOC = B // N_CORES          # images per core
SR = 8                        # strip rows (output rows per strip)
NSTRIP = H // SR
CUTOFF_L = 90.0
BN_EPS = 1e-5
NTOT = B * H * W              # BN stat count per channel
WP = W + 2                    # padded width

_CACHE = {}


def _emit_body(nc, tc, pools_tag, dram_io, mode='full'):
    import concourse.bass as bass  # noqa: F401
    from concourse import mybir

    F32 = mybir.dt.float32
    F16 = mybir.dt.float16
    AF = mybir.ActivationFunctionType
    x_d, wtp_d, wts_d, dmh_d, cb_d, gam_d, bet_d, out_d = dram_io

    from contextlib import ExitStack
    ctx = ExitStack()
    with ctx:
        persist = ctx.enter_context(tc.tile_pool(name=f"persist{pools_tag}", bufs=1))
        t_p = ctx.enter_context(tc.tile_pool(name=f"tt{pools_tag}", bufs=2))
        fq_p = ctx.enter_context(tc.tile_pool(name=f"fq{pools_tag}", bufs=4))
        dm_p = ctx.enter_context(tc.tile_pool(name=f"dm{pools_tag}", bufs=2))
        psum_p = ctx.enter_context(tc.tile_pool(name=f"ps{pools_tag}", bufs=2, space="PSUM"))
        out_p = ctx.enter_context(tc.tile_pool(name=f"out{pools_tag}", bufs=2))
        dram_p = ctx.enter_context(tc.tile_pool(name=f"dram{pools_tag}", bufs=1, space="DRAM"))

        # ---- persistent tiles ----
        y_sb = persist.tile([128, H * W // SR // 4 * NSTRIP], F16)  # [128, 65536] f16
        assert y_sb.shape[1] == B_LOC * C * H * W // 128
        stats6 = persist.tile([128, NSTRIP * 4, 6], F32)
        mv_t = persist.tile([128, 2], F32)
        msq128 = persist.tile([128, 1], F32)
        wtp_sb = persist.tile([128, 3, C], F16)   # tap pairs (dy, dx=0|1)
        wts_sb = persist.tile([64, 3, C], F16)    # tap singles (dy, dx=2)
        cb_t = persist.tile([64, 1], F32)
        gam_t = persist.tile([64, 1], F32)
        bet_t = persist.tile([64, 1], F32)
        eps_t = persist.tile([64, 1], F32)
        stats_sb = persist.tile([128, 2], F32)
        t0_t = persist.tile([64, 2], F32)
        t1_t = persist.tile([64, 2], F32)
        msq_t = persist.tile([64, 1], F32)
        var_t = persist.tile([64, 1], F32)
        tmp_t = persist.tile([64, 1], F32)
        s2_full = persist.tile([128, 1], F32)
        b2_full = persist.tile([128, 1], F32)

        # ---- load weights (f16 from host), per-channel params ----
        nc.sync.dma_start(out=wtp_sb[:, :, :], in_=wtp_d[:, :, :])
        nc.sync.dma_start(out=wts_sb[:, :, :], in_=wts_d[:, :, :])
        nc.sync.dma_start(out=cb_t[:, :], in_=cb_d[:, :])
        nc.sync.dma_start(out=gam_t[:, :], in_=gam_d[:, :])
        nc.sync.dma_start(out=bet_t[:, :], in_=bet_d[:, :])
        nc.vector.memset(eps_t[:, :], BN_EPS)

        # ---- pass 1: freq = x*dm, conv via paired taps, y -> SBUF f16, stats ----
        for s in range(NSTRIP):
            r0 = s * SR
            rlo = max(r0 - 1, 0)
            rhi = min(r0 + SR + 1, H)
            nrows = rhi - rlo
            b_lo = rlo - (r0 - 1)     # first buffer row filled

            T = t_p.tile([128, 2, SR + 2, WP], F16, tag="tt")
            dm_t = dm_p.tile([64, SR + 2, WP], F16, tag="dm")

            # load x rows (f16) into interior cols, both images on parts 0-63
            for img in range(B_LOC):
                nc.sync.dma_start(
                    out=T[0:64, img, b_lo:b_lo + nrows, 1:W + 1],
                    in_=x_d[img, :, rlo:rhi, :],
                )
            # halo rows at image edges: zero before the mul
            if s == 0:
                nc.gpsimd.memset(T[0:64, :, 0:1, :], 0.0)
            if s == NSTRIP - 1:
                nc.gpsimd.memset(T[0:64, :, SR + 1:SR + 2, :], 0.0)
            # clear pad columns every strip so all read bytes are written
            # in-generation (keeps the race detector's dep tracking exact)
            nc.gpsimd.memset(T[0:64, :, :, 0:1], 0.0)
            nc.gpsimd.memset(T[0:64, :, :, W + 1:W + 2], 0.0)

            # dm strip (f16, padded width, zero pad cols), broadcast to 64 parts
            nc.sync.dma_start(
                out=dm_t[:, b_lo:b_lo + nrows, :],
                in_=dmh_d[rlo:rhi, :].unsqueeze(0).to_broadcast((64, nrows, WP)),
            )
            if s == 0:
                # halo row of dm is junk (maybe NaN); make it finite so
                # 0*dm stays 0
                nc.gpsimd.memset(dm_t[:, 0:1, :], 0.0)
            if s == NSTRIP - 1:
                nc.gpsimd.memset(dm_t[:, SR + 1:SR + 2, :], 0.0)

            # freq = x * dm in place (f16); pads/halos come out 0 because
            # dm pads are 0 and x halos are 0
            for img in range(B_LOC):
                nc.vector.tensor_tensor(
                    out=T[0:64, img, :, :], in0=T[0:64, img, :, :],
                    in1=dm_t[:, :, :], op=mybir.AluOpType.mult,
                )

            # freq -> output channels [C:2C]: pool f16->f32 copy, then plain
            # HWDGE DMA (keeps every DMA a simple sync copy)
            for img in range(B_LOC):
                fq = fq_p.tile([64, SR, W], F32, tag="fq")
                nc.gpsimd.tensor_copy(fq[:, :, :], T[0:64, img, 1:SR + 1, 1:W + 1])
                nc.sync.dma_start(
                    out=out_d[img, C:2 * C, r0:r0 + SR, :],
                    in_=fq[:, :, :],
                )

            # upper partition half = both images shifted left one column
            nc.sync.dma_start(out=T[64:128, :, :, 0:W + 1],
                              in_=T[0:64, :, :, 1:W + 2])

            # conv: SR/2 rounds of N=512 (2 output rows); 6 matmul slots per
            # (round, img): 3 K=128 tap-pairs + 3 K=64 singles; the two imgs
            # run as col-tile pairs (0,0)/(0,64) into separate PSUM halves
            ps = psum_p.tile([128, SR // 2 * 512], mybir.dt.float32, tag="ps")
            for rnd in range(SR // 2):
                outs = []
                for img in range(B_LOC):
                    p0 = img * 64
                    outs.append(ps[p0:p0 + 64, rnd * 512:(rnd + 1) * 512]
                                .rearrange("p (a b) -> p a b", a=2))
                for dy in range(3):
                    rr = 2 * rnd + dy
                    for img in range(B_LOC):
                        nc.tensor.matmul(
                            outs[img],
                            wtp_sb[:, dy, :],
                            T[:, img, rr:rr + 2, 0:W],
                            start=(dy == 0), stop=False,
                            tile_position=(0, img * 64),
                            skip_group_check=True,
                        )
                for dy in range(3):
                    rr = 2 * rnd + dy
                    for img in range(B_LOC):
                        nc.tensor.matmul(
                            outs[img],
                            wts_sb[:, dy, :],
                            T[0:64, img, rr:rr + 2, 2:W + 2],
                            start=False, stop=(dy == 2),
                            tile_position=(0, img * 64),
                            skip_group_check=True,
                        )

            # y chunk -> SBUF f16; per-chunk BN stats from exact fp32 PSUM
            y_chunk = y_sb[:, s * (SR // 2 * 512):(s + 1) * (SR // 2 * 512)]
            nc.scalar.activation(out=y_chunk, in_=ps[:, :], func=AF.Copy)
            for j in range(SR // 2):
                nc.vector.bn_stats(
                    out=stats6[:, s * (SR // 2) + j, :],
                    in_=y_chunk[:, j * 512:(j + 1) * 512],
                )

        # ---- BN stats: aggregate, AllReduce across cores, finalize ----
        NPART = H * W  # y elements per partition on this core
        nc.vector.bn_aggr(out=mv_t[:, :], in_=stats6[:, :, :])  # (mean, var) per part
        # S1 = mean*n ; S2 = (var + mean^2)*n
        nc.vector.tensor_mul(msq128[:, :], mv_t[:, 0:1], mv_t[:, 0:1])
        nc.vector.tensor_add(stats_sb[:, 1:2], mv_t[:, 1:2], msq128[:, :])
        nc.scalar.mul(out=stats_sb[:, 1:2], in_=stats_sb[:, 1:2], mul=float(NPART))
        nc.scalar.mul(out=stats_sb[:, 0:1], in_=mv_t[:, 0:1], mul=float(NPART))
        ar_in = dram_p.tile([128, 2], F32)
        ar_out = dram_p.tile([128, 2], F32)
        nc.sync.dma_start(out=ar_in[:, :], in_=stats_sb[:, :])
        if mode == "nocoll":
            # analysis-only stand-in: keeps engine timeline shape without a
            # collective so single-core TimelineSim can run
            nc.sync.dma_start(out=ar_out[:, :], in_=ar_in[:, :])
        else:
            nc.gpsimd.collective_compute(
                "AllReduce", mybir.AluOpType.add,
                replica_groups=[list(range(N_CORES))],
                ins=[ar_in.opt()], outs=[ar_out.opt()],
            )
        nc.sync.dma_start(out=t0_t[:, :], in_=ar_out[0:64, :])
        nc.sync.dma_start(out=t1_t[:, :], in_=ar_out[64:128, :])
        nc.vector.tensor_add(t0_t[:, :], t0_t[:, :], t1_t[:, :])
        nc.scalar.mul(out=t0_t[:, :], in_=t0_t[:, :], mul=1.0 / NTOT)  # (mean, E[y^2])
        mean_ap = t0_t[:, 0:1]
        e2_ap = t0_t[:, 1:2]
        nc.vector.tensor_mul(msq_t[:, :], mean_ap, mean_ap)
        nc.vector.tensor_sub(var_t[:, :], e2_ap, msq_t[:, :])
        nc.scalar.activation(out=var_t[:, :], in_=var_t[:, :], func=AF.Sqrt,
                             bias=eps_t[:, :], scale=1.0)
        nc.vector.reciprocal(out=var_t[:, :], in_=var_t[:, :])  # rstd
        nc.vector.tensor_mul(s2_full[0:64, :], var_t[:, :], gam_t[:, :])
        # stats are over pre-bias y, so the conv bias cancels:
        # b2 = beta - mean_pre * s2
        nc.vector.tensor_mul(tmp_t[:, :], mean_ap, s2_full[0:64, :])
        nc.vector.tensor_sub(b2_full[0:64, :], bet_t[:, :], tmp_t[:, :])
        nc.sync.dma_start(out=s2_full[64:128, :], in_=s2_full[0:64, :])
        nc.sync.dma_start(out=b2_full[64:128, :], in_=b2_full[0:64, :])

        # ---- pass 2: act = SiLU(y * s2 + b2) -> output channels [0:C] ----
        CHUNK = 1024                       # 4 output rows per chunk
        n_chunks = (B_LOC * C * H * W // 128) // CHUNK
        for k in range(n_chunks):
            r0 = k * (CHUNK // W)  # 4 output rows per chunk
            o_t = out_p.tile([128, CHUNK], F32, tag="o")
            # CoreSim has no Silu executor; nocoll (sim-only) uses Tanh so the
            # data plumbing can be numerics-checked end to end
            nc.scalar.activation(
                out=o_t[:, :], in_=y_sb[:, k * CHUNK:(k + 1) * CHUNK],
                func=(AF.Tanh if mode == "nocoll" else AF.Silu),
                scale=s2_full[:, 0:1], bias=b2_full[:, 0:1],
            )
            for img in range(B_LOC):
                nc.sync.dma_start(
                    out=out_d[img, 0:C, r0:r0 + 4, :],
                    in_=o_t[img * 64:(img + 1) * 64, :],
                )


def _build(repeat=1, mode="full"):
    key = ("nc", repeat, mode)
    if key in _CACHE:
        return _CACHE[key]
    import concourse.bacc as bacc
    import concourse.tile as tile
    from concourse import mybir

    F32 = mybir.dt.float32
    F16 = mybir.dt.float16

    nc = bacc.Bacc("TRN2", target_bir_lowering=False, debug=False, num_devices=N_CORES)
    x_d = nc.dram_tensor("x", [B_LOC, C, H, W], F16, kind="ExternalInput")
    wtp_d = nc.dram_tensor("wtp", [128, 3, C], F16, kind="ExternalInput")
    wts_d = nc.dram_tensor("wts", [64, 3, C], F16, kind="ExternalInput")
    dmh_d = nc.dram_tensor("dmh", [H, WP], F16, kind="ExternalInput")
    cb_d = nc.dram_tensor("cb", [C, 1], F32, kind="ExternalInput")
    gam_d = nc.dram_tensor("gamma_in", [C, 1], F32, kind="ExternalInput")
    bet_d = nc.dram_tensor("beta_in", [C, 1], F32, kind="ExternalInput")
    out_d = nc.dram_tensor("out", [B_LOC, 2 * C, H, W], F32, kind="ExternalOutput")
    dram_io = (x_d, wtp_d, wts_d, dmh_d, cb_d, gam_d, bet_d, out_d)

    with tile.TileContext(nc) as tc:
        for rep in range(repeat):
            _emit_body(nc, tc, rep, dram_io, mode=mode)
    nc.compile()
    _CACHE[key] = nc
    return nc


def _host_inputs(x, conv_w, conv_b, gamma, beta):
    # dm exactly as the reference builds it (fp32), then f16 + zero pad cols
    u = (np.arange(H, dtype=np.float32) - H // 2)[:, None]
    v = (np.arange(W, dtype=np.float32) - W // 2)[None, :]
    d = np.sqrt(u * u + v * v)
    d = np.where(d == 0, np.float32(1e-6), d)
    filt = 1.0 / (1.0 + (d / np.float32(CUTOFF_L)) ** 2)
    dm = (0.8 * filt + 0.5).astype(np.float32)
    dmh = np.zeros((H, WP), np.float16)
    dmh[:, 1:W + 1] = dm.astype(np.float16)

    # wt[cin, tap, cout]; tap = 3*dy + dx
    wt = np.transpose(conv_w, (1, 2, 3, 0)).reshape(C, 9, C)
    wtp = np.empty((128, 3, C), np.float16)
    wts = np.empty((64, 3, C), np.float16)
    for dy in range(3):
        wtp[0:64, dy, :] = wt[:, 3 * dy + 0, :]
        wtp[64:128, dy, :] = wt[:, 3 * dy + 1, :]
        wts[:, dy, :] = wt[:, 3 * dy + 2, :]

    shared = {
        "wtp": wtp,
        "wts": wts,
        "dmh": dmh,
        "cb": conv_b.reshape(C, 1).astype(np.float32),
        "gamma_in": gamma.reshape(C, 1).astype(np.float32),
        "beta_in": beta.reshape(C, 1).astype(np.float32),
    }
    in_maps = []
    for c in range(N_CORES):
        m = dict(shared)
        m["x"] = np.ascontiguousarray(x[c * B_LOC:(c + 1) * B_LOC]).astype(np.float16)
        in_maps.append(m)
    return in_maps


def kernel(x, conv_w, conv_b, gamma, beta):
    from concourse.bass_utils import run_bass_kernel_spmd

    x = np.asarray(x)
    nc = _build(repeat=1)
    in_maps = _host_inputs(x, np.asarray(conv_w), np.asarray(conv_b),
                           np.asarray(gamma), np.asarray(beta))
    res = run_bass_kernel_spmd(nc, in_maps, core_ids=list(range(N_CORES)))
    out = np.concatenate([res.results[c]["out"] for c in range(N_CORES)], axis=0)
    return out.astype(np.float32)


# revision 26
# speedup vs baseline: 1.1777x; 1.1777x over previous
"""Trainium2 Bass kernel for: freq-domain Butterworth mask -> 3x3 conv ->
BatchNorm(train) -> SiLU, concat(act, freq).

Sharding: data-parallel over batch (2 images per core, 8 cores). BN statistics
are all-reduced across cores with an in-kernel AllReduce collective.

Layout: x arrives f16 (host-cast). Both local images live on SBUF partitions
0-63 with an image slot in the free dim: x2[0:64, i] = img i channels
(unshifted); one SBUF->SBUF DMA fills x2[64:128, i] = img i shifted left one
column. One K=128 matmul then computes TWO conv taps (dx=0 lower rows + dx=1
upper rows), and the two images run as concurrent column-tile matmuls
(tile_position (0,0)/(0,64)) into separate PSUM partition halves:
9 taps/round -> 6 matmul slots/round. Conv runs in f16 (full PE rate, fp32
PSUM accumulate). Conv output y stays resident in SBUF as f16 between the
stats pass and the normalization pass; BN stats use bn_stats/bn_aggr and one
tiny AllReduce. freq = x*dm is written out via a pool f16->f32 copy and a
plain HWDGE DMA (~1e-3 rel, well under the tolerance).
"""

import numpy as np

B, C, H, W = 16, 64, 256, 256
N_CORES = 8
B_LOC = B // N_CORES          # images per core
SR = 8                        # strip rows (output rows per strip)
NSTRIP = H // SR
CUTOFF_L = 90.0
BN_EPS = 1e-5
NTOT = B * H * W              # BN stat count per channel
WP = W + 2                    # padded width

_CACHE = {}


def _emit_body(nc, tc, pools_tag, dram_io, mode='full'):
    import concourse.bass as bass  # noqa: F401
    from concourse import mybir

    F32 = mybir.dt.float32
    F16 = mybir.dt.float16
    AF = mybir.ActivationFunctionType
    x_d, wtp_d, wts_d, dmh_d, cb_d, gam_d, bet_d, out_d = dram_io

    from contextlib import ExitStack
    ctx = ExitStack()
    with ctx:
        persist = ctx.enter_context(tc.tile_pool(name=f"persist{pools_tag}", bufs=1))
        t_p = ctx.enter_context(tc.tile_pool(name=f"tt{pools_tag}", bufs=2))
        fq_p = ctx.enter_context(tc.tile_pool(name=f"fq{pools_tag}", bufs=4))
        dm_p = ctx.enter_context(tc.tile_pool(name=f"dm{pools_tag}", bufs=2))
        psum_p = ctx.enter_context(tc.tile_pool(name=f"ps{pools_tag}", bufs=2, space="PSUM"))
        out_p = ctx.enter_context(tc.tile_pool(name=f"out{pools_tag}", bufs=2))
        dram_p = ctx.enter_context(tc.tile_pool(name=f"dram{pools_tag}", bufs=1, space="DRAM"))

        # ---- persistent tiles ----
        y_sb = persist.tile([128, H * W // SR // 4 * NSTRIP], F16)  # [128, 65536] f16
        assert y_sb.shape[1] == B_LOC * C * H * W // 128
        stats6 = persist.tile([128, NSTRIP * 4, 6], F32)
        mv_t = persist.tile([128, 2], F32)
        msq128 = persist.tile([128, 1], F32)
        wtp_sb = persist.tile([128, 3, C], F16)   # tap pairs (dy, dx=0|1)
        wts_sb = persist.tile([64, 3, C], F16)    # tap singles (dy, dx=2)
        cb_t = persist.tile([64, 1], F32)
        gam_t = persist.tile([64, 1], F32)
        bet_t = persist.tile([64, 1], F32)
        eps_t = persist.tile([64, 1], F32)
        stats_sb = persist.tile([128, 2], F32)
        t0_t = persist.tile([64, 2], F32)
        t1_t = persist.tile([64, 2], F32)
        msq_t = persist.tile([64, 1], F32)
        var_t = persist.tile([64, 1], F32)
        tmp_t = persist.tile([64, 1], F32)
        s2_full = persist.tile([128, 1], F32)
        b2_full = persist.tile([128, 1], F32)

        # ---- load weights (f16 from host), per-channel params ----
        nc.sync.dma_start(out=wtp_sb[:, :, :], in_=wtp_d[:, :, :])
        nc.sync.dma_start(out=wts_sb[:, :, :], in_=wts_d[:, :, :])
        nc.sync.dma_start(out=cb_t[:, :], in_=cb_d[:, :])
        nc.sync.dma_start(out=gam_t[:, :], in_=gam_d[:, :])
        nc.sync.dma_start(out=bet_t[:, :], in_=bet_d[:, :])
        nc.vector.memset(eps_t[:, :], BN_EPS)

        # ---- pass 1: freq = x*dm, conv via paired taps, y -> SBUF f16, stats ----
        for s in range(NSTRIP):
            r0 = s * SR
            rlo = max(r0 - 1, 0)
            rhi = min(r0 + SR + 1, H)
            nrows = rhi - rlo
            b_lo = rlo - (r0 - 1)     # first buffer row filled

            T = t_p.tile([128, 2, SR + 2, WP], F16, tag="tt")
            dm_t = dm_p.tile([64, SR + 2, WP], F16, tag="dm")

            # GPSIMD memset does 8-byte RMW; f16 rows are 516B (not 8-aligned)
            # so a pad-column memset concurrent with the x DMA can restore
            # stale neighbor bytes. Never memset concurrently with a DMA into
            # the same tile: clear the whole lower half on first buffer use
            # (the DMA overlaps the cleared range -> Tile orders it after),
            # and rely on the in-place multiply to regenerate zero pads every
            # strip (dm's pad columns are zero, halos are zero).
            if s < 2:
                nc.gpsimd.memset(T[0:64, :, :, :], 0.0)
                nc.gpsimd.memset(dm_t[:, :, :], 0.0)
            if s == NSTRIP - 1:
                # bottom halo row (row SR+1 never DMA'd this strip): extend
                # the clear into row SR so it overlaps the DMA -> ordered
                nc.gpsimd.memset(T[0:64, :, SR:SR + 2, :], 0.0)
                nc.gpsimd.memset(dm_t[:, SR:SR + 2, :], 0.0)

            # load x rows (f16) into interior cols, both images on parts 0-63
            for img in range(B_LOC):
                nc.sync.dma_start(
                    out=T[0:64, img, b_lo:b_lo + nrows, 1:W + 1],
                    in_=x_d[img, :, rlo:rhi, :],
                )

            # dm strip (f16, padded width, zero pad cols), broadcast to 64 parts
            nc.sync.dma_start(
                out=dm_t[:, b_lo:b_lo + nrows, :],
                in_=dmh_d[rlo:rhi, :].unsqueeze(0).to_broadcast((64, nrows, WP)),
            )

            # freq = x * dm in place (f16); pads/halos come out 0 because
            # dm pads are 0 and x halos are 0
            for img in range(B_LOC):
                nc.vector.tensor_tensor(
                    out=T[0:64, img, :, :], in0=T[0:64, img, :, :],
                    in1=dm_t[:, :, :], op=mybir.AluOpType.mult,
                )

            # freq -> output channels [C:2C]: f16->f32 engine copy (ACT for
            # img0, DVE for img1 — gpsimd is half-rate on 64 partitions),
            # then plain HWDGE DMA
            for img in range(B_LOC):
                fq = fq_p.tile([64, SR, W], F32, tag="fq")
                src = T[0:64, img, 1:SR + 1, 1:W + 1]
                if img == 0:
                    nc.scalar.activation(out=fq[:, :, :], in_=src, func=AF.Copy)
                else:
                    nc.vector.tensor_copy(fq[:, :, :], src)
                nc.sync.dma_start(
                    out=out_d[img, C:2 * C, r0:r0 + SR, :],
                    in_=fq[:, :, :],
                )

            # upper partition half = both images shifted left one column
            nc.sync.dma_start(out=T[64:128, :, :, 0:W + 1],
                              in_=T[0:64, :, :, 1:W + 2])

            # conv: SR/2 rounds of N=512 (2 output rows); 6 matmul slots per
            # (round, img): 3 K=128 tap-pairs + 3 K=64 singles; the two imgs
            # run as col-tile pairs (0,0)/(0,64) into separate PSUM halves
            ps = psum_p.tile([128, SR // 2 * 512], mybir.dt.float32, tag="ps")
            for rnd in range(SR // 2):
                outs = []
                for img in range(B_LOC):
                    p0 = img * 64
                    outs.append(ps[p0:p0 + 64, rnd * 512:(rnd + 1) * 512]
                                .rearrange("p (a b) -> p a b", a=2))
                for dy in range(3):
                    rr = 2 * rnd + dy
                    for img in range(B_LOC):
                        nc.tensor.matmul(
                            outs[img],
                            wtp_sb[:, dy, :],
                            T[:, img, rr:rr + 2, 0:W],
                            start=(dy == 0), stop=False,
                            tile_position=(0, img * 64),
                            skip_group_check=True,
                        )
                for dy in range(3):
                    rr = 2 * rnd + dy
                    for img in range(B_LOC):
                        nc.tensor.matmul(
                            outs[img],
                            wts_sb[:, dy, :],
                            T[0:64, img, rr:rr + 2, 2:W + 2],
                            start=False, stop=(dy == 2),
                            tile_position=(0, img * 64),
                            skip_group_check=True,
                        )

            # y chunk -> SBUF f16; per-chunk BN stats from exact fp32 PSUM
            y_chunk = y_sb[:, s * (SR // 2 * 512):(s + 1) * (SR // 2 * 512)]
            nc.scalar.activation(out=y_chunk, in_=ps[:, :], func=AF.Copy)
            for j in range(SR // 2):
                nc.vector.bn_stats(
                    out=stats6[:, s * (SR // 2) + j, :],
                    in_=y_chunk[:, j * 512:(j + 1) * 512],
                )

        # ---- BN stats: aggregate, AllReduce across cores, finalize ----
        NPART = H * W  # y elements per partition on this core
        nc.vector.bn_aggr(out=mv_t[:, :], in_=stats6[:, :, :])  # (mean, var) per part
        # S1 = mean*n ; S2 = (var + mean^2)*n
        nc.vector.tensor_mul(msq128[:, :], mv_t[:, 0:1], mv_t[:, 0:1])
        nc.vector.tensor_add(stats_sb[:, 1:2], mv_t[:, 1:2], msq128[:, :])
        nc.scalar.mul(out=stats_sb[:, 1:2], in_=stats_sb[:, 1:2], mul=float(NPART))
        nc.scalar.mul(out=stats_sb[:, 0:1], in_=mv_t[:, 0:1], mul=float(NPART))
        ar_in = dram_p.tile([128, 2], F32)
        ar_out = dram_p.tile([128, 2], F32)
        nc.sync.dma_start(out=ar_in[:, :], in_=stats_sb[:, :])
        if mode == "nocoll":
            # analysis-only stand-in: keeps engine timeline shape without a
            # collective so single-core TimelineSim can run
            nc.sync.dma_start(out=ar_out[:, :], in_=ar_in[:, :])
        else:
            nc.gpsimd.collective_compute(
                "AllReduce", mybir.AluOpType.add,
                replica_groups=[list(range(N_CORES))],
                ins=[ar_in.opt()], outs=[ar_out.opt()],
            )
        nc.sync.dma_start(out=t0_t[:, :], in_=ar_out[0:64, :])
        nc.sync.dma_start(out=t1_t[:, :], in_=ar_out[64:128, :])
        nc.vector.tensor_add(t0_t[:, :], t0_t[:, :], t1_t[:, :])
        nc.scalar.mul(out=t0_t[:, :], in_=t0_t[:, :], mul=1.0 / NTOT)  # (mean, E[y^2])
        mean_ap = t0_t[:, 0:1]
        e2_ap = t0_t[:, 1:2]
        nc.vector.tensor_mul(msq_t[:, :], mean_ap, mean_ap)
        nc.vector.tensor_sub(var_t[:, :], e2_ap, msq_t[:, :])
        nc.scalar.activation(out=var_t[:, :], in_=var_t[:, :], func=AF.Sqrt,
                             bias=eps_t[:, :], scale=1.0)
        nc.vector.reciprocal(out=var_t[:, :], in_=var_t[:, :])  # rstd
        nc.vector.tensor_mul(s2_full[0:64, :], var_t[:, :], gam_t[:, :])
        # stats are over pre-bias y, so the conv bias cancels:
        # b2 = beta - mean_pre * s2
        nc.vector.tensor_mul(tmp_t[:, :], mean_ap, s2_full[0:64, :])
        nc.vector.tensor_sub(b2_full[0:64, :], bet_t[:, :], tmp_t[:, :])
        nc.sync.dma_start(out=s2_full[64:128, :], in_=s2_full[0:64, :])
        nc.sync.dma_start(out=b2_full[64:128, :], in_=b2_full[0:64, :])

        # ---- pass 2: act = SiLU(y * s2 + b2) -> output channels [0:C] ----
        CHUNK = 1024                       # 4 output rows per chunk
        n_chunks = (B_LOC * C * H * W // 128) // CHUNK
        for k in range(n_chunks):
            r0 = k * (CHUNK // W)  # 4 output rows per chunk
            o_t = out_p.tile([128, CHUNK], F32, tag="o")
            # CoreSim has no Silu executor; nocoll (sim-only) uses Tanh so the
            # data plumbing can be numerics-checked end to end
            nc.scalar.activation(
                out=o_t[:, :], in_=y_sb[:, k * CHUNK:(k + 1) * CHUNK],
                func=(AF.Tanh if mode == "nocoll" else AF.Silu),
                scale=s2_full[:, 0:1], bias=b2_full[:, 0:1],
            )
            for img in range(B_LOC):
                nc.sync.dma_start(
                    out=out_d[img, 0:C, r0:r0 + 4, :],
                    in_=o_t[img * 64:(img + 1) * 64, :],
                )


def _build(repeat=1, mode="full"):
    key = ("nc", repeat, mode)
    if key in _CACHE:
        return _CACHE[key]
    import concourse.bacc as bacc
    import concourse.tile as tile
    from concourse import mybir

    F32 = mybir.dt.float32
    F16 = mybir.dt.float16

    nc = bacc.Bacc("TRN2", target_bir_lowering=False, debug=False, num_devices=N_CORES)
    x_d = nc.dram_tensor("x", [B_LOC, C, H, W], F16, kind="ExternalInput")
    wtp_d = nc.dram_tensor("wtp", [128, 3, C], F16, kind="ExternalInput")
    wts_d = nc.dram_tensor("wts", [64, 3, C], F16, kind="ExternalInput")
    dmh_d = nc.dram_tensor("dmh", [H, WP], F16, kind="ExternalInput")
    cb_d = nc.dram_tensor("cb", [C, 1], F32, kind="ExternalInput")
    gam_d = nc.dram_tensor("gamma_in", [C, 1], F32, kind="ExternalInput")
    bet_d = nc.dram_tensor("beta_in", [C, 1], F32, kind="ExternalInput")
    out_d = nc.dram_tensor("out", [B_LOC, 2 * C, H, W], F32, kind="ExternalOutput")
    dram_io = (x_d, wtp_d, wts_d, dmh_d, cb_d, gam_d, bet_d, out_d)

    with tile.TileContext(nc) as tc:
        for rep in range(repeat):
            _emit_body(nc, tc, rep, dram_io, mode=mode)
    nc.compile()
    _CACHE[key] = nc
    return nc


def _host_inputs(x, conv_w, conv_b, gamma, beta):
    # dm exactly as the reference builds it (fp32), then f16 + zero pad cols
    u = (np.arange(H, dtype=np.float32) - H // 2)[:, None]
    v = (np.arange(W, dtype=np.float32) - W // 2)[None, :]
    d = np.sqrt(u * u + v * v)
    d = np.where(d == 0, np.float32(1e-6), d)
    filt = 1.0 / (1.0 + (d / np.float32(CUTOFF_L)) ** 2)
    dm = (0.8 * filt + 0.5).astype(np.float32)
    dmh = np.zeros((H, WP), np.float16)
    dmh[:, 1:W + 1] = dm.astype(np.float16)

    # wt[cin, tap, cout]; tap = 3*dy + dx
    wt = np.transpose(conv_w, (1, 2, 3, 0)).reshape(C, 9, C)
    wtp = np.empty((128, 3, C), np.float16)
    wts = np.empty((64, 3, C), np.float16)
    for dy in range(3):
        wtp[0:64, dy, :] = wt[:, 3 * dy + 0, :]
        wtp[64:128, dy, :] = wt[:, 3 * dy + 1, :]
        wts[:, dy, :] = wt[:, 3 * dy + 2, :]

    shared = {
        "wtp": wtp,
        "wts": wts,
        "dmh": dmh,
        "cb": conv_b.reshape(C, 1).astype(np.float32),
        "gamma_in": gamma.reshape(C, 1).astype(np.float32),
        "beta_in": beta.reshape(C, 1).astype(np.float32),
    }
    in_maps = []
    for c in range(N_CORES):
        m = dict(shared)
        m["x"] = np.ascontiguousarray(x[c * B_LOC:(c + 1) * B_LOC]).astype(np.float16)
        in_maps.append(m)
    return in_maps


def kernel(x, conv_w, conv_b, gamma, beta):
    from concourse.bass_utils import run_bass_kernel_spmd

    x = np.asarray(x)
    nc = _build(repeat=1)
    in_maps = _host_inputs(x, np.asarray(conv_w), np.asarray(conv_b),
                           np.asarray(gamma), np.asarray(beta))
    res = run_bass_kernel_spmd(nc, in_maps, core_ids=list(range(N_CORES)))
    out = np.concatenate([res.results[c]["out"] for c in range(N_CORES)], axis=0)
    return out.astype(np.float32)
